# revision 9
# baseline (speedup 1.0000x reference)
"""CBOW negative-sampling loss kernel for 8 TRN2 NeuronCores.

Strategy (data-parallel, per sharding hint):
  - Shard the batch (B=16384) across 8 cores -> 2048 rows/core.
  - Per core the embedding tables are COMPACTED on host: only the
    distinct rows this core's lookups touch (<=22528, worst case) are
    uploaded, relabelled 0..n-1, padded to 32768 rows, bf16.  This
    more than halves HBM footprint vs replicating the full f32 tables
    and lets indices fit int16.
  - The 43008 row-gathers per core are split between the two SWDGE
    descriptor-generation paths, which run CONCURRENTLY on different
    queues:
      * qPoolDynamic(0):  classic indirect_dma_start, one index per
        partition per instruction (~1.5us / 128 rows measured).
      * qPoolDynamic1..3: batched dma_gather ucode (int16 index list,
        ~25ns/row single-queue, ~2.3x faster across 3 queues).
    Both are slot-exact (dma_gather places list element j at partition
    j%128, block j//128), so the compute is identical for all tiles
    and the target always lands at candidate 0.
  - DVE computes per-tile context sums (add tree) and the 11 dot
    products per row (mult + segmented reduce) in bf16; ACT applies
    sigmoid to ALL scores with scale -0.1 and a single ln(+eps) with
    free-dim accumulation.  The target's positive term is recovered on
    host via log sig(x) - log sig(-x) = x, i.e. loss row-sum =
    sum_c log(sig(-s_c/10)+eps) + s_pos/10.
"""

import os

import numpy as np

import concourse.bacc as bacc
import concourse.bass as bass
import concourse.mybir as mybir
import concourse.tile as tile
from concourse.bass_utils import run_bass_kernel_spmd

VOCAB = 100000
DIM = 128
B = 16384
CWIN = 10
K = 10
EPS = 1e-9
NCORES = 8
P = 128
BPC = B // NCORES            # 2048 batch rows per core
NTILES = BPC // P            # 16 tiles of 128 rows
NIDX = CWIN + 1 + K          # 21 lookups per batch row
CTAB = 32768                 # compacted table rows (per core, per table)

F32 = mybir.dt.float32
BF16 = mybir.dt.bfloat16
I16 = mybir.dt.int16
I32 = mybir.dt.int32
MULT = mybir.AluOpType.mult
ADD = mybir.AluOpType.add
AX_X = mybir.AxisListType.X
SIGMOID = mybir.ActivationFunctionType.Sigmoid
LN = mybir.ActivationFunctionType.Ln

# ---- tunables -----------------------------------------------------------
NG_TILES = int(os.environ.get("KCFG_NG", "16"))  # tiles via dma_gather
GCHUNK = int(os.environ.get("KCFG_GCHUNK", "2")) # tiles per dma_gather inst
NQUEUES = int(os.environ.get("KCFG_NQUEUES", "1"))
GATHER_BUFS = int(os.environ.get("KCFG_GBUFS", "4"))
IND_BUFS = int(os.environ.get("KCFG_IBUFS", "3"))

NI_TILES = NTILES - NG_TILES                     # tiles via indirect path
assert NG_TILES % GCHUNK == 0
NGC = NG_TILES // GCHUNK                         # dma_gather chunk count
CTX_NI = GCHUNK * CWIN * P                       # idx per ctx gather (2560)
TN_NI = GCHUNK * (K + 1) * P                     # idx per tn gather (2816)
CW16 = CTX_NI // 16
TW16 = TN_NI // 16
CHUNK16 = CW16 + TW16


def build_kernel_body(tc, idx32, idx16, ctab_in, ctab_out, usum):
    """Emit the per-core program.

    idx32: [P, max(NI_TILES,1)*NIDX] int32. For indirect tile u (global
           tile t = NG_TILES+u), cols u*21+j: j<10 ctx lookups
           (compacted in_emb ids), j>=10 target+negatives (compacted
           out_emb ids; j==10 is the target).
    idx16: [P, max(NGC,1)*CHUNK16] int16 dma_gather wrapped lists per
           chunk: ctx list (CTX_NI) then tn list (TN_NI); list elem j
           -> partition j%128, block j//128; blocks ordered
           (tile-in-chunk, slot), target at slot 0 of each tn group.
    usum:  [P, 2] f32; col 0 = sum over all tiles/candidates of
           log(sigmoid(-s/10)+eps); col 1 = sum over tiles of raw
           target score s_pos.
    """
    nc = tc.nc
    with (
        tc.tile_pool(name="io", bufs=1) as io_pool,
        tc.tile_pool(name="g", bufs=GATHER_BUFS) as gpool,
        tc.tile_pool(name="ind", bufs=IND_BUFS) as ipool,
        tc.tile_pool(name="work", bufs=2) as wpool,
    ):
        idx32_t = io_pool.tile([P, max(NI_TILES, 1) * NIDX], I32)
        if NI_TILES:
            nc.sync.dma_start(out=idx32_t[:], in_=idx32[:, :])
        idx16_t = io_pool.tile([P, max(NGC, 1) * CHUNK16], I16)
        if NGC:
            nc.sync.dma_start(out=idx16_t[:], in_=idx16[:, :])

        eps_t = io_pool.tile([P, 1], F32)
        nc.vector.memset(eps_t[:], EPS)

        # staging for all scores; col t*11+c = candidate c of tile t
        s_all = io_pool.tile([P, NTILES * (K + 1)], F32)
        us = io_pool.tile([P, 2], F32)

        def compute_tile(t_idx, ctx_ap, tn_ap):
            a1 = wpool.tile([P, 5 * DIM], BF16, tag="a1")
            nc.vector.tensor_add(
                a1[:], ctx_ap[:, 0 : 5 * DIM], ctx_ap[:, 5 * DIM : 10 * DIM]
            )
            b1 = wpool.tile([P, 2 * DIM], BF16, tag="b1")
            nc.vector.tensor_add(
                b1[:], a1[:, 0 : 2 * DIM], a1[:, 2 * DIM : 4 * DIM]
            )
            cs = wpool.tile([P, DIM], BF16, tag="cs")
            nc.vector.tensor_add(cs[:], b1[:, 0:DIM], b1[:, DIM : 2 * DIM])
            nc.vector.tensor_add(cs[:], cs[:], a1[:, 4 * DIM : 5 * DIM])

            prod = wpool.tile([P, (K + 1) * DIM], BF16, tag="prod")
            prod3 = prod[:].rearrange("p (k d) -> p k d", d=DIM)
            tn3 = tn_ap.rearrange("p (k d) -> p k d", d=DIM)
            cs_b = cs[:][:, None, :].to_broadcast([P, K + 1, DIM])
            nc.vector.tensor_tensor(prod3, tn3, cs_b, MULT)
            nc.vector.tensor_reduce(
                out=s_all[:, t_idx * (K + 1) : (t_idx + 1) * (K + 1)],
                in_=prod3, axis=AX_X, op=ADD,
            )

        # ---- dma_gather tiles: persistent buffers, all gathers issued
        # back-to-back (no WAR stalls on Pool), single queue; DVE
        # trails on per-instruction semaphores.
        if NG_TILES:
            ctx_all = io_pool.tile([P, NG_TILES * CWIN * DIM], BF16)
            tn_all = io_pool.tile([P, NG_TILES * (K + 1) * DIM], BF16)
            for c in range(NGC):
                q = 1 + (c % (NQUEUES - 1)) if NQUEUES > 1 else 0
                base16 = c * CHUNK16
                cbase = c * GCHUNK * CWIN * DIM
                tbase = c * GCHUNK * (K + 1) * DIM
                nc.gpsimd.dma_gather(
                    out_ap=ctx_all[:, cbase : cbase + GCHUNK * CWIN * DIM]
                    .rearrange("p (q d) -> p q d", d=DIM),
                    in_ap=ctab_in[:, :],
                    idxs_ap=idx16_t[:, base16 : base16 + CW16],
                    num_idxs=CTX_NI,
                    num_idxs_reg=CTX_NI,
                    elem_size=DIM,
                    single_packet=False,
                    queue_num=q,
                )
                nc.gpsimd.dma_gather(
                    out_ap=tn_all[:, tbase : tbase + GCHUNK * (K + 1) * DIM]
                    .rearrange("p (q d) -> p q d", d=DIM),
                    in_ap=ctab_out[:, :],
                    idxs_ap=idx16_t[:, base16 + CW16 : base16 + CHUNK16],
                    num_idxs=TN_NI,
                    num_idxs_reg=TN_NI,
                    elem_size=DIM,
                    single_packet=False,
                    queue_num=q,
                )
            for t in range(NG_TILES):
                compute_tile(
                    t,
                    ctx_all[:, t * CWIN * DIM : (t + 1) * CWIN * DIM],
                    tn_all[:, t * (K + 1) * DIM : (t + 1) * (K + 1) * DIM],
                )

        # ---- indirect tiles, queue 0
        for u in range(NI_TILES):
            c0 = u * NIDX
            ctx_g = ipool.tile([P, CWIN * DIM], BF16, tag="ictx")
            for j in range(CWIN):
                nc.gpsimd.indirect_dma_start(
                    out=ctx_g[:, j * DIM : (j + 1) * DIM],
                    out_offset=None,
                    in_=ctab_in[:, :],
                    in_offset=bass.IndirectOffsetOnAxis(
                        ap=idx32_t[:, c0 + j : c0 + j + 1], axis=0
                    ),
                )
            tn_g = ipool.tile([P, (K + 1) * DIM], BF16, tag="itn")
            for j in range(K + 1):
                nc.gpsimd.indirect_dma_start(
                    out=tn_g[:, j * DIM : (j + 1) * DIM],
                    out_offset=None,
                    in_=ctab_out[:, :],
                    in_offset=bass.IndirectOffsetOnAxis(
                        ap=idx32_t[:, c0 + CWIN + j : c0 + CWIN + j + 1],
                        axis=0,
                    ),
                )
            compute_tile(NG_TILES + u, ctx_g[:], tn_g[:])

        # ---- batched activation phases (one table load each) ---------
        sig = io_pool.tile([P, NTILES * (K + 1)], F32)
        nc.scalar.activation(sig[:], s_all[:], SIGMOID, scale=-0.1)
        lnv = io_pool.tile([P, NTILES * (K + 1)], F32)
        nc.scalar.activation(
            lnv[:], sig[:], LN, bias=eps_t[:], accum_out=us[:, 0:1]
        )
        # sum of raw target scores (candidate 0 of each tile)
        spos = (
            s_all[:]
            .rearrange("p (t c) -> p t c", c=K + 1)[:, :, 0:1]
            .rearrange("p t c -> p (t c)")
        )
        nc.vector.tensor_reduce(out=us[:, 1:2], in_=spos, axis=AX_X, op=ADD)

        nc.sync.dma_start(out=usum[:, :], in_=us[:])


def build_nc():
    nc = bacc.Bacc(
        "TRN2",
        target_bir_lowering=False,
        debug=False,
        enable_asserts=False,
        num_devices=NCORES,
        num_swdge_queues=NQUEUES,
    )
    idx32 = nc.dram_tensor(
        "idx32", [P, max(NI_TILES, 1) * NIDX], I32, kind="ExternalInput"
    )
    idx16 = nc.dram_tensor(
        "idx16", [P, max(NGC, 1) * CHUNK16], I16, kind="ExternalInput"
    )
    ctab_in = nc.dram_tensor("ctab_in", [CTAB, DIM], BF16,
                             kind="ExternalInput")
    ctab_out = nc.dram_tensor("ctab_out", [CTAB, DIM], BF16,
                              kind="ExternalInput")
    usum = nc.dram_tensor("usum", [P, 2], F32, kind="ExternalOutput")
    with tile.TileContext(nc) as tc:
        build_kernel_body(tc, idx32.ap(), idx16.ap(), ctab_in.ap(),
                          ctab_out.ap(), usum.ap())
    nc.compile()
    return nc


def _wrap16(arr):
    """flat index list -> [128, n/16] int16 dma_gather layout."""
    w = arr.reshape(-1, 16).T
    return np.tile(w, (8, 1)).astype(np.int16)


def make_in_maps(context, target, negatives, in_emb, out_emb):
    context = np.asarray(context).astype(np.int64)
    target = np.asarray(target).astype(np.int64)
    negatives = np.asarray(negatives).astype(np.int64)
    in_emb = np.asarray(in_emb, dtype=np.float32)
    out_emb = np.asarray(out_emb, dtype=np.float32)
    bf16 = mybir.dt.np(BF16)
    tn_full = np.concatenate([target[:, None], negatives], axis=1)  # [B, 11]
    in_maps = []
    for c in range(NCORES):
        sl = slice(c * BPC, (c + 1) * BPC)
        # [P, NTILES, slots] index cubes (partition = batch row % 128)
        ctx_t = (
            context[sl].reshape(NTILES, P, CWIN).transpose(1, 0, 2)
        )  # [P, T, 10]
        tn_t = (
            tn_full[sl].reshape(NTILES, P, K + 1).transpose(1, 0, 2)
        )  # [P, T, 11], slot 0 = target

        # per-(core, table) compaction; <=20480/22528 distinct always
        uin, cin = np.unique(ctx_t, return_inverse=True)
        cin = cin.reshape(ctx_t.shape).astype(np.int32)
        uout, ctn = np.unique(tn_t, return_inverse=True)
        ctn = ctn.reshape(tn_t.shape).astype(np.int32)
        assert len(uin) <= CTAB and len(uout) <= CTAB
        ctab_in = np.zeros((CTAB, DIM), dtype=bf16)
        ctab_in[: len(uin)] = in_emb[uin].astype(bf16)
        ctab_out = np.zeros((CTAB, DIM), dtype=bf16)
        ctab_out[: len(uout)] = out_emb[uout].astype(bf16)

        # int16 dma_gather lists: chunk c covers tiles [2c, 2c+1];
        # list elem j -> (partition j%128, block j//128), block =
        # (tile_in_chunk, slot) -> order (b, w, p) when flattening
        parts16 = []
        for ch in range(NGC):
            t0 = ch * GCHUNK
            ctx_list = (
                cin[:, t0 : t0 + GCHUNK, :].transpose(1, 2, 0).reshape(-1)
            )
            tn_list = (
                ctn[:, t0 : t0 + GCHUNK, :].transpose(1, 2, 0).reshape(-1)
            )
            parts16.append(_wrap16(ctx_list))
            parts16.append(_wrap16(tn_list))
        if parts16:
            idx16 = np.ascontiguousarray(np.concatenate(parts16, axis=1))
        else:
            idx16 = np.zeros((P, CHUNK16), np.int16)

        # int32 indirect columns for tiles NG_TILES..15
        if NI_TILES:
            cols = np.concatenate(
                [cin[:, NG_TILES:, :], ctn[:, NG_TILES:, :]], axis=2
            )  # [P, NI_TILES, 21]
            idx32 = np.ascontiguousarray(
                cols.reshape(P, NI_TILES * NIDX).astype(np.int32)
            )
        else:
            idx32 = np.zeros((P, NIDX), np.int32)

        in_maps.append(
            {
                "idx32": idx32,
                "idx16": idx16,
                "ctab_in": ctab_in,
                "ctab_out": ctab_out,
            }
        )
    return in_maps


_NC_CACHE = []
LAST_RESULT = None  # BassKernelResults of the most recent run (for profiling)


def kernel(**inputs) -> np.ndarray:
    global LAST_RESULT
    in_maps = make_in_maps(
        inputs["context"],
        inputs["target"],
        inputs["negatives"],
        inputs["in_emb"],
        inputs["out_emb"],
    )
    if not _NC_CACHE:
        _NC_CACHE.append(build_nc())
    nc = _NC_CACHE[0]
    res = run_bass_kernel_spmd(nc, in_maps, core_ids=list(range(NCORES)))
    LAST_RESULT = res
    total = 0.0
    for r in res.results:
        u = r["usum"].astype(np.float64)
        total += u[:, 0].sum() + 0.1 * u[:, 1].sum()
    return np.array(-total / B, dtype=np.float32)


# revision 10
# speedup vs baseline: 1.5129x; 1.5129x over previous
"""CBOW negative-sampling loss kernel for 8 TRN2 NeuronCores.

Strategy (data-parallel, per sharding hint):
  - Shard the batch (B=16384) across 8 cores -> 2048 rows/core.
  - Per core the embedding tables are COMPACTED on host: only the
    distinct rows this core's lookups touch (<=22528, worst case) are
    uploaded, relabelled 0..n-1, padded to 32768 rows, bf16.  This
    more than halves HBM footprint vs replicating the full f32 tables
    and lets indices fit int16.
  - The 43008 row-gathers per core are split between the two SWDGE
    descriptor-generation paths, which run CONCURRENTLY on different
    queues:
      * qPoolDynamic(0):  classic indirect_dma_start, one index per
        partition per instruction (~1.5us / 128 rows measured).
      * qPoolDynamic1..3: batched dma_gather ucode (int16 index list,
        ~25ns/row single-queue, ~2.3x faster across 3 queues).
    Both are slot-exact (dma_gather places list element j at partition
    j%128, block j//128), so the compute is identical for all tiles
    and the target always lands at candidate 0.
  - DVE computes per-tile context sums (add tree) and the 11 dot
    products per row (mult + segmented reduce) in bf16; ACT applies
    sigmoid to ALL scores with scale -0.1 and a single ln(+eps) with
    free-dim accumulation.  The target's positive term is recovered on
    host via log sig(x) - log sig(-x) = x, i.e. loss row-sum =
    sum_c log(sig(-s_c/10)+eps) + s_pos/10.
"""

import os

import numpy as np

import concourse.bacc as bacc
import concourse.bass as bass
import concourse.mybir as mybir
import concourse.tile as tile
from concourse.bass_utils import run_bass_kernel_spmd

VOCAB = 100000
DIM = 128
B = 16384
CWIN = 10
K = 10
EPS = 1e-9
NCORES = 8
P = 128
BPC = B // NCORES            # 2048 batch rows per core
NTILES = BPC // P            # 16 tiles of 128 rows
NIDX = CWIN + 1 + K          # 21 lookups per batch row
CTAB = 32768                 # compacted table rows (per core, per table)

F32 = mybir.dt.float32
BF16 = mybir.dt.bfloat16
I16 = mybir.dt.int16
I32 = mybir.dt.int32
MULT = mybir.AluOpType.mult
ADD = mybir.AluOpType.add
AX_X = mybir.AxisListType.X
SIGMOID = mybir.ActivationFunctionType.Sigmoid
LN = mybir.ActivationFunctionType.Ln

# ---- tunables -----------------------------------------------------------
NG_TILES = int(os.environ.get("KCFG_NG", "16"))  # tiles via dma_gather
GCHUNK = int(os.environ.get("KCFG_GCHUNK", "2")) # tiles per dma_gather inst
NQUEUES = int(os.environ.get("KCFG_NQUEUES", "1"))
GATHER_BUFS = int(os.environ.get("KCFG_GBUFS", "4"))
IND_BUFS = int(os.environ.get("KCFG_IBUFS", "3"))

NI_TILES = NTILES - NG_TILES                     # tiles via indirect path
assert NG_TILES % GCHUNK == 0
NGC = NG_TILES // GCHUNK                         # dma_gather chunk count
CTX_NI = GCHUNK * CWIN * P                       # idx per ctx gather (2560)
TN_NI = GCHUNK * (K + 1) * P                     # idx per tn gather (2816)
CW16 = CTX_NI // 16
TW16 = TN_NI // 16
CHUNK16 = CW16 + TW16


def build_kernel_body(tc, idx32, idx16, ctab_in, ctab_out, usum):
    """Emit the per-core program.

    idx32: [P, max(NI_TILES,1)*NIDX] int32. For indirect tile u (global
           tile t = NG_TILES+u), cols u*21+j: j<10 ctx lookups
           (compacted in_emb ids), j>=10 target+negatives (compacted
           out_emb ids; j==10 is the target).
    idx16: [P, max(NGC,1)*CHUNK16] int16 dma_gather wrapped lists per
           chunk: ctx list (CTX_NI) then tn list (TN_NI); list elem j
           -> partition j%128, block j//128; blocks ordered
           (tile-in-chunk, slot), target at slot 0 of each tn group.
    usum:  [P, 2] f32; col 0 = sum over all tiles/candidates of
           log(sigmoid(-s/10)+eps); col 1 = sum over tiles of raw
           target score s_pos.
    """
    nc = tc.nc
    with (
        tc.tile_pool(name="io", bufs=1) as io_pool,
        tc.tile_pool(name="g", bufs=GATHER_BUFS) as gpool,
        tc.tile_pool(name="ind", bufs=IND_BUFS) as ipool,
        tc.tile_pool(name="work", bufs=2) as wpool,
    ):
        idx32_t = io_pool.tile([P, max(NI_TILES, 1) * NIDX], I32)
        if NI_TILES:
            nc.sync.dma_start(out=idx32_t[:], in_=idx32[:, :])
        idx16_t = io_pool.tile([P, max(NGC, 1) * CHUNK16], I16)
        if NGC:
            nc.sync.dma_start(out=idx16_t[:], in_=idx16[:, :])

        eps_t = io_pool.tile([P, 1], F32)
        nc.vector.memset(eps_t[:], EPS)

        # staging for all scores; col t*11+c = candidate c of tile t
        s_all = io_pool.tile([P, NTILES * (K + 1)], F32)
        us = io_pool.tile([P, 2], F32)

        def compute_tile(t_idx, ctx_ap, tn_ap):
            a1 = wpool.tile([P, 5 * DIM], BF16, tag="a1")
            nc.vector.tensor_add(
                a1[:], ctx_ap[:, 0 : 5 * DIM], ctx_ap[:, 5 * DIM : 10 * DIM]
            )
            b1 = wpool.tile([P, 2 * DIM], BF16, tag="b1")
            nc.vector.tensor_add(
                b1[:], a1[:, 0 : 2 * DIM], a1[:, 2 * DIM : 4 * DIM]
            )
            cs = wpool.tile([P, DIM], BF16, tag="cs")
            nc.vector.tensor_add(cs[:], b1[:, 0:DIM], b1[:, DIM : 2 * DIM])
            nc.vector.tensor_add(cs[:], cs[:], a1[:, 4 * DIM : 5 * DIM])

            prod = wpool.tile([P, (K + 1) * DIM], BF16, tag="prod")
            prod3 = prod[:].rearrange("p (k d) -> p k d", d=DIM)
            tn3 = tn_ap.rearrange("p (k d) -> p k d", d=DIM)
            cs_b = cs[:][:, None, :].to_broadcast([P, K + 1, DIM])
            nc.vector.tensor_tensor(prod3, tn3, cs_b, MULT)
            nc.vector.tensor_reduce(
                out=s_all[:, t_idx * (K + 1) : (t_idx + 1) * (K + 1)],
                in_=prod3, axis=AX_X, op=ADD,
            )

        # ---- dma_gather tiles (chunks of GCHUNK); pooled buffers so
        # successive gathers have no write-write edges, DVE trails.
        for c in range(NGC):
            q = 1 + (c % (NQUEUES - 1)) if NQUEUES > 1 else 0
            base16 = c * CHUNK16
            ctx_g = gpool.tile([P, GCHUNK * CWIN * DIM], BF16, tag="gctx")
            nc.gpsimd.dma_gather(
                out_ap=ctx_g[:].rearrange("p (q d) -> p q d", d=DIM),
                in_ap=ctab_in[:, :],
                idxs_ap=idx16_t[:, base16 : base16 + CW16],
                num_idxs=CTX_NI,
                num_idxs_reg=CTX_NI,
                elem_size=DIM,
                single_packet=False,
                queue_num=q,
            )
            tn_g = gpool.tile([P, GCHUNK * (K + 1) * DIM], BF16, tag="gtn")
            nc.gpsimd.dma_gather(
                out_ap=tn_g[:].rearrange("p (q d) -> p q d", d=DIM),
                in_ap=ctab_out[:, :],
                idxs_ap=idx16_t[:, base16 + CW16 : base16 + CHUNK16],
                num_idxs=TN_NI,
                num_idxs_reg=TN_NI,
                elem_size=DIM,
                single_packet=False,
                queue_num=q,
            )
            for b in range(GCHUNK):
                compute_tile(
                    c * GCHUNK + b,
                    ctx_g[:, b * CWIN * DIM : (b + 1) * CWIN * DIM],
                    tn_g[:, b * (K + 1) * DIM : (b + 1) * (K + 1) * DIM],
                )

        # ---- indirect tiles, queue 0
        for u in range(NI_TILES):
            c0 = u * NIDX
            ctx_g = ipool.tile([P, CWIN * DIM], BF16, tag="ictx")
            for j in range(CWIN):
                nc.gpsimd.indirect_dma_start(
                    out=ctx_g[:, j * DIM : (j + 1) * DIM],
                    out_offset=None,
                    in_=ctab_in[:, :],
                    in_offset=bass.IndirectOffsetOnAxis(
                        ap=idx32_t[:, c0 + j : c0 + j + 1], axis=0
                    ),
                )
            tn_g = ipool.tile([P, (K + 1) * DIM], BF16, tag="itn")
            for j in range(K + 1):
                nc.gpsimd.indirect_dma_start(
                    out=tn_g[:, j * DIM : (j + 1) * DIM],
                    out_offset=None,
                    in_=ctab_out[:, :],
                    in_offset=bass.IndirectOffsetOnAxis(
                        ap=idx32_t[:, c0 + CWIN + j : c0 + CWIN + j + 1],
                        axis=0,
                    ),
                )
            compute_tile(NG_TILES + u, ctx_g[:], tn_g[:])

        # ---- batched activation phases (one table load each) ---------
        sig = io_pool.tile([P, NTILES * (K + 1)], F32)
        nc.scalar.activation(sig[:], s_all[:], SIGMOID, scale=-0.1)
        lnv = io_pool.tile([P, NTILES * (K + 1)], F32)
        nc.scalar.activation(
            lnv[:], sig[:], LN, bias=eps_t[:], accum_out=us[:, 0:1]
        )
        # sum of raw target scores (candidate 0 of each tile)
        spos = (
            s_all[:]
            .rearrange("p (t c) -> p t c", c=K + 1)[:, :, 0:1]
            .rearrange("p t c -> p (t c)")
        )
        nc.vector.tensor_reduce(out=us[:, 1:2], in_=spos, axis=AX_X, op=ADD)

        nc.sync.dma_start(out=usum[:, :], in_=us[:])


def build_nc():
    nc = bacc.Bacc(
        "TRN2",
        target_bir_lowering=False,
        debug=False,
        enable_asserts=False,
        num_devices=NCORES,
        num_swdge_queues=NQUEUES,
    )
    idx32 = nc.dram_tensor(
        "idx32", [P, max(NI_TILES, 1) * NIDX], I32, kind="ExternalInput"
    )
    idx16 = nc.dram_tensor(
        "idx16", [P, max(NGC, 1) * CHUNK16], I16, kind="ExternalInput"
    )
    ctab_in = nc.dram_tensor("ctab_in", [CTAB, DIM], BF16,
                             kind="ExternalInput")
    ctab_out = nc.dram_tensor("ctab_out", [CTAB, DIM], BF16,
                              kind="ExternalInput")
    usum = nc.dram_tensor("usum", [P, 2], F32, kind="ExternalOutput")
    with tile.TileContext(nc) as tc:
        build_kernel_body(tc, idx32.ap(), idx16.ap(), ctab_in.ap(),
                          ctab_out.ap(), usum.ap())
    nc.compile()
    return nc


def _wrap16(arr):
    """flat index list -> [128, n/16] int16 dma_gather layout."""
    w = arr.reshape(-1, 16).T
    return np.tile(w, (8, 1)).astype(np.int16)


def make_in_maps(context, target, negatives, in_emb, out_emb):
    context = np.asarray(context).astype(np.int64)
    target = np.asarray(target).astype(np.int64)
    negatives = np.asarray(negatives).astype(np.int64)
    in_emb = np.asarray(in_emb, dtype=np.float32)
    out_emb = np.asarray(out_emb, dtype=np.float32)
    bf16 = mybir.dt.np(BF16)
    tn_full = np.concatenate([target[:, None], negatives], axis=1)  # [B, 11]
    in_maps = []
    for c in range(NCORES):
        sl = slice(c * BPC, (c + 1) * BPC)
        # [P, NTILES, slots] index cubes (partition = batch row % 128)
        ctx_t = (
            context[sl].reshape(NTILES, P, CWIN).transpose(1, 0, 2)
        )  # [P, T, 10]
        tn_t = (
            tn_full[sl].reshape(NTILES, P, K + 1).transpose(1, 0, 2)
        )  # [P, T, 11], slot 0 = target

        # per-(core, table) compaction; <=20480/22528 distinct always
        uin, cin = np.unique(ctx_t, return_inverse=True)
        cin = cin.reshape(ctx_t.shape).astype(np.int32)
        uout, ctn = np.unique(tn_t, return_inverse=True)
        ctn = ctn.reshape(tn_t.shape).astype(np.int32)
        assert len(uin) <= CTAB and len(uout) <= CTAB
        ctab_in = np.zeros((CTAB, DIM), dtype=bf16)
        ctab_in[: len(uin)] = in_emb[uin].astype(bf16)
        ctab_out = np.zeros((CTAB, DIM), dtype=bf16)
        ctab_out[: len(uout)] = out_emb[uout].astype(bf16)

        # int16 dma_gather lists: chunk c covers tiles [2c, 2c+1];
        # list elem j -> (partition j%128, block j//128), block =
        # (tile_in_chunk, slot) -> order (b, w, p) when flattening
        parts16 = []
        for ch in range(NGC):
            t0 = ch * GCHUNK
            ctx_list = (
                cin[:, t0 : t0 + GCHUNK, :].transpose(1, 2, 0).reshape(-1)
            )
            tn_list = (
                ctn[:, t0 : t0 + GCHUNK, :].transpose(1, 2, 0).reshape(-1)
            )
            parts16.append(_wrap16(ctx_list))
            parts16.append(_wrap16(tn_list))
        if parts16:
            idx16 = np.ascontiguousarray(np.concatenate(parts16, axis=1))
        else:
            idx16 = np.zeros((P, CHUNK16), np.int16)

        # int32 indirect columns for tiles NG_TILES..15
        if NI_TILES:
            cols = np.concatenate(
                [cin[:, NG_TILES:, :], ctn[:, NG_TILES:, :]], axis=2
            )  # [P, NI_TILES, 21]
            idx32 = np.ascontiguousarray(
                cols.reshape(P, NI_TILES * NIDX).astype(np.int32)
            )
        else:
            idx32 = np.zeros((P, NIDX), np.int32)

        in_maps.append(
            {
                "idx32": idx32,
                "idx16": idx16,
                "ctab_in": ctab_in,
                "ctab_out": ctab_out,
            }
        )
    return in_maps


_NC_CACHE = []
LAST_RESULT = None  # BassKernelResults of the most recent run (for profiling)


def kernel(**inputs) -> np.ndarray:
    global LAST_RESULT
    in_maps = make_in_maps(
        inputs["context"],
        inputs["target"],
        inputs["negatives"],
        inputs["in_emb"],
        inputs["out_emb"],
    )
    if not _NC_CACHE:
        _NC_CACHE.append(build_nc())
    nc = _NC_CACHE[0]
    res = run_bass_kernel_spmd(nc, in_maps, core_ids=list(range(NCORES)))
    LAST_RESULT = res
    total = 0.0
    for r in res.results:
        u = r["usum"].astype(np.float64)
        total += u[:, 0].sum() + 0.1 * u[:, 1].sum()
    return np.array(-total / B, dtype=np.float32)


# revision 11
# speedup vs baseline: 1.6086x; 1.0632x over previous
"""CBOW negative-sampling loss kernel for 8 TRN2 NeuronCores.

Strategy (data-parallel, per sharding hint):
  - Shard the batch (B=16384) across 8 cores -> 2048 rows/core.
  - Per core the embedding tables are COMPACTED on host: only the
    distinct rows this core's lookups touch (<=22528, worst case) are
    uploaded, relabelled 0..n-1, padded to 32768 rows, bf16.  This
    more than halves HBM footprint vs replicating the full f32 tables
    and lets indices fit int16.
  - The 43008 row-gathers per core are split between the two SWDGE
    descriptor-generation paths, which run CONCURRENTLY on different
    queues:
      * qPoolDynamic(0):  classic indirect_dma_start, one index per
        partition per instruction (~1.5us / 128 rows measured).
      * qPoolDynamic1..3: batched dma_gather ucode (int16 index list,
        ~25ns/row single-queue, ~2.3x faster across 3 queues).
    Both are slot-exact (dma_gather places list element j at partition
    j%128, block j//128), so the compute is identical for all tiles
    and the target always lands at candidate 0.
  - DVE computes per-tile context sums (add tree) and the 11 dot
    products per row (mult + segmented reduce) in bf16; ACT applies
    sigmoid to ALL scores with scale -0.1 and a single ln(+eps) with
    free-dim accumulation.  The target's positive term is recovered on
    host via log sig(x) - log sig(-x) = x, i.e. loss row-sum =
    sum_c log(sig(-s_c/10)+eps) + s_pos/10.
"""

import os

import numpy as np

import concourse.bacc as bacc
import concourse.bass as bass
import concourse.mybir as mybir
import concourse.tile as tile
from concourse.bass_utils import run_bass_kernel_spmd

VOCAB = 100000
DIM = 128
B = 16384
CWIN = 10
K = 10
EPS = 1e-9
NCORES = 8
P = 128
BPC = B // NCORES            # 2048 batch rows per core
NTILES = BPC // P            # 16 tiles of 128 rows
NIDX = CWIN + 1 + K          # 21 lookups per batch row
CTAB = 32768                 # compacted table rows (per core, per table)

F32 = mybir.dt.float32
BF16 = mybir.dt.bfloat16
I16 = mybir.dt.int16
I32 = mybir.dt.int32
MULT = mybir.AluOpType.mult
ADD = mybir.AluOpType.add
AX_X = mybir.AxisListType.X
SIGMOID = mybir.ActivationFunctionType.Sigmoid
LN = mybir.ActivationFunctionType.Ln

# ---- tunables -----------------------------------------------------------
# Measured on HW: the qPoolDynamic descriptor-generation ucode is the
# bottleneck (~1.3us serial per 128-row indirect instruction; the
# batched dma_gather ucode is no faster per row in-kernel and mixing
# the two paths interferes).  All-indirect measured best.
NG_TILES = int(os.environ.get("KCFG_NG", "0"))   # tiles via dma_gather
GCHUNK = int(os.environ.get("KCFG_GCHUNK", "2")) # tiles per dma_gather inst
NQUEUES = int(os.environ.get("KCFG_NQUEUES", "4"))
GATHER_BUFS = int(os.environ.get("KCFG_GBUFS", "4"))
IND_BUFS = int(os.environ.get("KCFG_IBUFS", "3"))

NI_TILES = NTILES - NG_TILES                     # tiles via indirect path
assert NG_TILES % GCHUNK == 0
NGC = NG_TILES // GCHUNK                         # dma_gather chunk count
CTX_NI = GCHUNK * CWIN * P                       # idx per ctx gather (2560)
TN_NI = GCHUNK * (K + 1) * P                     # idx per tn gather (2816)
CW16 = CTX_NI // 16
TW16 = TN_NI // 16
CHUNK16 = CW16 + TW16


def build_kernel_body(tc, idx32, idx16, ctab_in, ctab_out, usum):
    """Emit the per-core program.

    idx32: [P, max(NI_TILES,1)*NIDX] int32. For indirect tile u (global
           tile t = NG_TILES+u), cols u*21+j: j<10 ctx lookups
           (compacted in_emb ids), j>=10 target+negatives (compacted
           out_emb ids; j==10 is the target).
    idx16: [P, max(NGC,1)*CHUNK16] int16 dma_gather wrapped lists per
           chunk: ctx list (CTX_NI) then tn list (TN_NI); list elem j
           -> partition j%128, block j//128; blocks ordered
           (tile-in-chunk, slot), target at slot 0 of each tn group.
    usum:  [P, 2] f32; col 0 = sum over all tiles/candidates of
           log(sigmoid(-s/10)+eps); col 1 = sum over tiles of raw
           target score s_pos.
    """
    nc = tc.nc
    with (
        tc.tile_pool(name="io", bufs=1) as io_pool,
        tc.tile_pool(name="g", bufs=GATHER_BUFS) as gpool,
        tc.tile_pool(name="ind", bufs=IND_BUFS) as ipool,
        tc.tile_pool(name="work", bufs=2) as wpool,
    ):
        idx32_t = io_pool.tile([P, max(NI_TILES, 1) * NIDX], I32)
        if NI_TILES:
            nc.sync.dma_start(out=idx32_t[:], in_=idx32[:, :])
        idx16_t = io_pool.tile([P, max(NGC, 1) * CHUNK16], I16)
        if NGC:
            nc.sync.dma_start(out=idx16_t[:], in_=idx16[:, :])

        eps_t = io_pool.tile([P, 1], F32)
        nc.vector.memset(eps_t[:], EPS)

        # staging for all scores; col t*11+c = candidate c of tile t
        s_all = io_pool.tile([P, NTILES * (K + 1)], F32)
        us = io_pool.tile([P, 2], F32)

        def compute_tile(t_idx, ctx_ap, tn_ap):
            a1 = wpool.tile([P, 5 * DIM], BF16, tag="a1")
            nc.vector.tensor_add(
                a1[:], ctx_ap[:, 0 : 5 * DIM], ctx_ap[:, 5 * DIM : 10 * DIM]
            )
            b1 = wpool.tile([P, 2 * DIM], BF16, tag="b1")
            nc.vector.tensor_add(
                b1[:], a1[:, 0 : 2 * DIM], a1[:, 2 * DIM : 4 * DIM]
            )
            cs = wpool.tile([P, DIM], BF16, tag="cs")
            nc.vector.tensor_add(cs[:], b1[:, 0:DIM], b1[:, DIM : 2 * DIM])
            nc.vector.tensor_add(cs[:], cs[:], a1[:, 4 * DIM : 5 * DIM])

            prod = wpool.tile([P, (K + 1) * DIM], BF16, tag="prod")
            prod3 = prod[:].rearrange("p (k d) -> p k d", d=DIM)
            tn3 = tn_ap.rearrange("p (k d) -> p k d", d=DIM)
            cs_b = cs[:][:, None, :].to_broadcast([P, K + 1, DIM])
            nc.vector.tensor_tensor(prod3, tn3, cs_b, MULT)
            nc.vector.tensor_reduce(
                out=s_all[:, t_idx * (K + 1) : (t_idx + 1) * (K + 1)],
                in_=prod3, axis=AX_X, op=ADD,
            )

        # ---- dma_gather tiles (chunks of GCHUNK); pooled buffers so
        # successive gathers have no write-write edges, DVE trails.
        for c in range(NGC):
            q = 1 + (c % (NQUEUES - 1)) if NQUEUES > 1 else 0
            base16 = c * CHUNK16
            ctx_g = gpool.tile([P, GCHUNK * CWIN * DIM], BF16, tag="gctx")
            nc.gpsimd.dma_gather(
                out_ap=ctx_g[:].rearrange("p (q d) -> p q d", d=DIM),
                in_ap=ctab_in[:, :],
                idxs_ap=idx16_t[:, base16 : base16 + CW16],
                num_idxs=CTX_NI,
                num_idxs_reg=CTX_NI,
                elem_size=DIM,
                single_packet=False,
                queue_num=q,
            )
            tn_g = gpool.tile([P, GCHUNK * (K + 1) * DIM], BF16, tag="gtn")
            nc.gpsimd.dma_gather(
                out_ap=tn_g[:].rearrange("p (q d) -> p q d", d=DIM),
                in_ap=ctab_out[:, :],
                idxs_ap=idx16_t[:, base16 + CW16 : base16 + CHUNK16],
                num_idxs=TN_NI,
                num_idxs_reg=TN_NI,
                elem_size=DIM,
                single_packet=False,
                queue_num=q,
            )
            for b in range(GCHUNK):
                compute_tile(
                    c * GCHUNK + b,
                    ctx_g[:, b * CWIN * DIM : (b + 1) * CWIN * DIM],
                    tn_g[:, b * (K + 1) * DIM : (b + 1) * (K + 1) * DIM],
                )

        # ---- indirect tiles, queue 0
        for u in range(NI_TILES):
            c0 = u * NIDX
            ctx_g = ipool.tile([P, CWIN * DIM], BF16, tag="ictx")
            for j in range(CWIN):
                nc.gpsimd.indirect_dma_start(
                    out=ctx_g[:, j * DIM : (j + 1) * DIM],
                    out_offset=None,
                    in_=ctab_in[:, :],
                    in_offset=bass.IndirectOffsetOnAxis(
                        ap=idx32_t[:, c0 + j : c0 + j + 1], axis=0
                    ),
                )
            tn_g = ipool.tile([P, (K + 1) * DIM], BF16, tag="itn")
            for j in range(K + 1):
                nc.gpsimd.indirect_dma_start(
                    out=tn_g[:, j * DIM : (j + 1) * DIM],
                    out_offset=None,
                    in_=ctab_out[:, :],
                    in_offset=bass.IndirectOffsetOnAxis(
                        ap=idx32_t[:, c0 + CWIN + j : c0 + CWIN + j + 1],
                        axis=0,
                    ),
                )
            compute_tile(NG_TILES + u, ctx_g[:], tn_g[:])

        # ---- batched activation phases (one table load each) ---------
        sig = io_pool.tile([P, NTILES * (K + 1)], F32)
        nc.scalar.activation(sig[:], s_all[:], SIGMOID, scale=-0.1)
        lnv = io_pool.tile([P, NTILES * (K + 1)], F32)
        nc.scalar.activation(
            lnv[:], sig[:], LN, bias=eps_t[:], accum_out=us[:, 0:1]
        )
        # sum of raw target scores (candidate 0 of each tile)
        spos = (
            s_all[:]
            .rearrange("p (t c) -> p t c", c=K + 1)[:, :, 0:1]
            .rearrange("p t c -> p (t c)")
        )
        nc.vector.tensor_reduce(out=us[:, 1:2], in_=spos, axis=AX_X, op=ADD)

        nc.sync.dma_start(out=usum[:, :], in_=us[:])


def build_nc():
    nc = bacc.Bacc(
        "TRN2",
        target_bir_lowering=False,
        debug=False,
        enable_asserts=False,
        num_devices=NCORES,
        num_swdge_queues=NQUEUES,
    )
    idx32 = nc.dram_tensor(
        "idx32", [P, max(NI_TILES, 1) * NIDX], I32, kind="ExternalInput"
    )
    idx16 = nc.dram_tensor(
        "idx16", [P, max(NGC, 1) * CHUNK16], I16, kind="ExternalInput"
    )
    ctab_in = nc.dram_tensor("ctab_in", [CTAB, DIM], BF16,
                             kind="ExternalInput")
    ctab_out = nc.dram_tensor("ctab_out", [CTAB, DIM], BF16,
                              kind="ExternalInput")
    usum = nc.dram_tensor("usum", [P, 2], F32, kind="ExternalOutput")
    with tile.TileContext(nc) as tc:
        build_kernel_body(tc, idx32.ap(), idx16.ap(), ctab_in.ap(),
                          ctab_out.ap(), usum.ap())
    nc.compile()
    return nc


def _wrap16(arr):
    """flat index list -> [128, n/16] int16 dma_gather layout."""
    w = arr.reshape(-1, 16).T
    return np.tile(w, (8, 1)).astype(np.int16)


def make_in_maps(context, target, negatives, in_emb, out_emb):
    context = np.asarray(context).astype(np.int64)
    target = np.asarray(target).astype(np.int64)
    negatives = np.asarray(negatives).astype(np.int64)
    in_emb = np.asarray(in_emb, dtype=np.float32)
    out_emb = np.asarray(out_emb, dtype=np.float32)
    bf16 = mybir.dt.np(BF16)
    tn_full = np.concatenate([target[:, None], negatives], axis=1)  # [B, 11]
    in_maps = []
    for c in range(NCORES):
        sl = slice(c * BPC, (c + 1) * BPC)
        # [P, NTILES, slots] index cubes (partition = batch row % 128)
        ctx_t = (
            context[sl].reshape(NTILES, P, CWIN).transpose(1, 0, 2)
        )  # [P, T, 10]
        tn_t = (
            tn_full[sl].reshape(NTILES, P, K + 1).transpose(1, 0, 2)
        )  # [P, T, 11], slot 0 = target

        # per-(core, table) compaction; <=20480/22528 distinct always
        uin, cin = np.unique(ctx_t, return_inverse=True)
        cin = cin.reshape(ctx_t.shape).astype(np.int32)
        uout, ctn = np.unique(tn_t, return_inverse=True)
        ctn = ctn.reshape(tn_t.shape).astype(np.int32)
        assert len(uin) <= CTAB and len(uout) <= CTAB
        ctab_in = np.zeros((CTAB, DIM), dtype=bf16)
        ctab_in[: len(uin)] = in_emb[uin].astype(bf16)
        ctab_out = np.zeros((CTAB, DIM), dtype=bf16)
        ctab_out[: len(uout)] = out_emb[uout].astype(bf16)

        # int16 dma_gather lists: chunk c covers tiles [2c, 2c+1];
        # list elem j -> (partition j%128, block j//128), block =
        # (tile_in_chunk, slot) -> order (b, w, p) when flattening
        parts16 = []
        for ch in range(NGC):
            t0 = ch * GCHUNK
            ctx_list = (
                cin[:, t0 : t0 + GCHUNK, :].transpose(1, 2, 0).reshape(-1)
            )
            tn_list = (
                ctn[:, t0 : t0 + GCHUNK, :].transpose(1, 2, 0).reshape(-1)
            )
            parts16.append(_wrap16(ctx_list))
            parts16.append(_wrap16(tn_list))
        if parts16:
            idx16 = np.ascontiguousarray(np.concatenate(parts16, axis=1))
        else:
            idx16 = np.zeros((P, CHUNK16), np.int16)

        # int32 indirect columns for tiles NG_TILES..15
        if NI_TILES:
            cols = np.concatenate(
                [cin[:, NG_TILES:, :], ctn[:, NG_TILES:, :]], axis=2
            )  # [P, NI_TILES, 21]
            idx32 = np.ascontiguousarray(
                cols.reshape(P, NI_TILES * NIDX).astype(np.int32)
            )
        else:
            idx32 = np.zeros((P, NIDX), np.int32)

        in_maps.append(
            {
                "idx32": idx32,
                "idx16": idx16,
                "ctab_in": ctab_in,
                "ctab_out": ctab_out,
            }
        )
    return in_maps


_NC_CACHE = []
LAST_RESULT = None  # BassKernelResults of the most recent run (for profiling)


def kernel(**inputs) -> np.ndarray:
    global LAST_RESULT
    in_maps = make_in_maps(
        inputs["context"],
        inputs["target"],
        inputs["negatives"],
        inputs["in_emb"],
        inputs["out_emb"],
    )
    if not _NC_CACHE:
        _NC_CACHE.append(build_nc())
    nc = _NC_CACHE[0]
    res = run_bass_kernel_spmd(nc, in_maps, core_ids=list(range(NCORES)))
    LAST_RESULT = res
    total = 0.0
    for r in res.results:
        u = r["usum"].astype(np.float64)
        total += u[:, 0].sum() + 0.1 * u[:, 1].sum()
    return np.array(-total / B, dtype=np.float32)
